# revision 14
# baseline (speedup 1.0000x reference)
"""Multi-head attention (B=4, S=2048, D=1024, H=16, causal) on 8 TRN2 NeuronCores.

Sharding: core i handles batch i//2 and head-group i%2 (8 heads / 512 projection
columns). Each core computes a partial output projection over its 512 rows of Wo;
the host sums the two partials per batch and adds bo. No device collectives.

Per-core dataflow (bf16 matmuls, fp32 softmax):
  QT/KT = W-stationary projections of pre-transposed x; V in natural layout with
  an interleaved ones column per head (softmax denominator rides the AV matmul).
  Scores are computed transposed [k, q] in 3-k-tile PSUM chunks; one wide ACT
  exp per chunk evicts to SBUF bf16; causal masking is a single multiply per
  diagonal k-tile against a host-provided mask; AV accumulates [out^T | denom];
  normalization uses a DMA-reshaped reciprocal ([1,512] -> [128,4] so the DVE
  divides 4 elements per lane instead of 512) and a GPSIMD partition broadcast.
"""

import sys

for _p in ("/opt/trn_rl_repo",):
    if _p not in sys.path:
        sys.path.insert(0, _p)

import numpy as np
import ml_dtypes

BF16 = ml_dtypes.bfloat16

B, S, D = 4, 2048, 1024
H, HD = 16, 64
HPC = H // 2          # heads per core: 8
DPC = D // 2          # projection cols per core: 512
NCORES = 8
SCALE = 1.0 / np.sqrt(np.float32(HD))
CH = 4                # k-tiles per score chunk (4 PSUM banks, single slot;
                      # the one-chunk AV delay keeps PE busy during exp)

_compiled = None


def _chunks(nkt):
    out, s = [], 0
    while s < nkt:
        n = min(CH, nkt - s)
        out.append((s, n))
        s += n
    return out


def _build():
    import concourse.bacc as bacc
    import concourse.mybir as mybir
    import concourse.tile as tile

    f32 = mybir.dt.float32
    bf = mybir.dt.bfloat16
    Exp = mybir.ActivationFunctionType.Exp
    Copy = mybir.ActivationFunctionType.Copy

    nc = bacc.Bacc("TRN2", target_bir_lowering=False, debug=False)

    xtq = nc.dram_tensor("xtq", [D, S], bf, kind="ExternalInput")
    xtk = nc.dram_tensor("xtk", [D, S], bf, kind="ExternalInput")
    xtv = nc.dram_tensor("xtv", [D, S], bf, kind="ExternalInput")
    wq = nc.dram_tensor("wq", [D, DPC], bf, kind="ExternalInput")
    wk = nc.dram_tensor("wk", [D, DPC], bf, kind="ExternalInput")
    wv = nc.dram_tensor("wv", [D, DPC], bf, kind="ExternalInput")
    wo = nc.dram_tensor("wo", [DPC, D], bf, kind="ExternalInput")
    bq = nc.dram_tensor("bq", [1, DPC], bf, kind="ExternalInput")
    bk = nc.dram_tensor("bk", [1, DPC], bf, kind="ExternalInput")
    bv = nc.dram_tensor("bv", [1, DPC], bf, kind="ExternalInput")
    dmask = nc.dram_tensor("dmask", [128, 2048], bf, kind="ExternalInput")
    y = nc.dram_tensor("y", [S, D], f32, kind="ExternalOutput")

    NKD = D // 128        # 8 contraction tiles for projections
    NST = S // 128        # 16 seq tiles
    NSB = S // 512        # 4 seq blocks
    NHP = HPC // 2        # 4 head pairs / 128-wide col groups

    with tile.TileContext(nc) as tc:
        with (
            tc.tile_pool(name="consts", bufs=1) as consts,
            tc.tile_pool(name="wqp", bufs=NKD) as wqp,
            tc.tile_pool(name="wkp", bufs=NKD) as wkp,
            tc.tile_pool(name="wvp", bufs=NKD) as wvp,
            tc.tile_pool(name="wop", bufs=4) as wop,
            tc.tile_pool(name="xt", bufs=2 * NKD) as xtp,
            tc.tile_pool(name="qt", bufs=NHP) as qtp,
            tc.tile_pool(name="kt", bufs=NHP) as ktp,
            tc.tile_pool(name="vp", bufs=NST) as vpool,
            tc.tile_pool(name="ex", bufs=2) as expool,
            tc.tile_pool(name="ot", bufs=NHP) as otp,
            tc.tile_pool(name="ys", bufs=2) as ysp,
            tc.tile_pool(name="rb", bufs=1) as rbp,
            tc.tile_pool(name="rc", bufs=1) as rcp,
            tc.tile_pool(name="ps", bufs=4, space="PSUM") as psp,
            tc.tile_pool(name="sc", bufs=1, space="PSUM") as scp,
        ):
            # constants
            dmt = consts.tile([128, 2048], bf, tag="dmt")
            nc.sync.dma_start(dmt[:], dmask.ap()[:])
            ones = consts.tile([1, 512], bf, tag="ones")
            nc.gpsimd.memset(ones[:], 1.0)
            bqt = consts.tile([1, DPC], bf, tag="bq")
            nc.sync.dma_start(bqt[:], bq.ap()[:])
            bkt = consts.tile([1, DPC], bf, tag="bk")
            nc.sync.dma_start(bkt[:], bk.ap()[:])
            bvt = consts.tile([1, DPC], bf, tag="bv")
            nc.sync.dma_start(bvt[:], bv.ap()[:])

            # weights: wv + xtv queued first so the first V matmul starts ASAP
            wvt = []
            for kd in range(NKD):
                w = wvp.tile([128, DPC], bf, name=f"wv{kd}", tag="wv")
                nc.sync.dma_start(w[:], wv.ap()[kd * 128:(kd + 1) * 128, :])
                wvt.append(w)
            vts = []
            xts = []
            for kd in range(NKD):
                xt = xtp.tile([128, S], bf, name=f"xt_v{kd}", tag="xt")
                nc.sync.dma_start(xt[:], xtv.ap()[kd * 128:(kd + 1) * 128, :])
                xts.append(xt)
            wqt, wkt = [], []
            for kd in range(NKD):
                for lst, pool, t in ((wqt, wqp, wq), (wkt, wkp, wk)):
                    w = pool.tile([128, DPC], bf)
                    nc.sync.dma_start(w[:], t.ap()[kd * 128:(kd + 1) * 128, :])
                    lst.append(w)
            wot = []
            for hp in range(4):
                w = wop.tile([128, D], bf)
                nc.sync.dma_start(w[:], wo.ap()[hp * 128:(hp + 1) * 128, :])
                wot.append(w)

            # ---- V projection (natural layout), interleaved [8 heads x 65] + ones col
            for st in range(NST):
                vt = vpool.tile([128, HPC * 65], bf)
                vts.append(vt)
                ps = psp.tile([128, 512], f32, tag="ps")
                for kd in range(NKD):
                    nc.tensor.matmul(
                        ps[:],
                        xts[kd][:, st * 128:(st + 1) * 128],
                        wvt[kd][:],
                        start=(kd == 0), stop=False,
                    )
                nc.tensor.matmul(ps[:], ones[0:1, 0:128], bvt[0:1, :],
                                 start=False, stop=True)
                v3 = vt[:].rearrange("p (h c) -> p h c", h=HPC, c=65)
                nc.scalar.activation(
                    v3[:, :, 0:64],
                    ps[:].rearrange("p (h c) -> p h c", h=HPC, c=64),
                    Copy,
                )
                nc.gpsimd.memset(v3[:, :, 64:65], 1.0)

            # ---- QT / KT projections. hp=0 is emitted up front; hp=1..3 are
            # queued as fine-grained filler steps woven between attention chunks
            # so the tensor engine never idles below the HAM busy threshold
            # while ACT paces the exp pipeline.
            qts, kts = [], []
            xtq_ts, xtk_ts = [], []
            for (src_t, xlist) in ((xtq, xtq_ts), (xtk, xtk_ts)):
                for kd in range(NKD):
                    xt = xtp.tile([128, S], bf, name=f"xt_{src_t.name}{kd}", tag="xt")
                    nc.sync.dma_start(xt[:], src_t.ap()[kd * 128:(kd + 1) * 128, :])
                    xlist.append(xt)
            for pool, lst, nm in ((qtp, qts, "qt"), (ktp, kts, "kt")):
                for hp in range(NHP):
                    lst.append(pool.tile([128, S], bf, name=f"{nm}{hp}", tag=nm))

            def proj_group_steps(xts, wts, bias, dest, hp, sb, on_act):
                ps_box = []
                def mk_mm(kd):
                    def step():
                        if kd == 0:
                            ps_box.append(psp.tile([128, 512], f32, name="psq", tag="ps"))
                        nc.tensor.matmul(
                            ps_box[0][:],
                            wts[kd][:, hp * 128:(hp + 1) * 128],
                            xts[kd][:, sb * 512:(sb + 1) * 512],
                            start=(kd == 0), stop=False,
                        )
                    return step
                def bias_step():
                    nc.tensor.matmul(
                        ps_box[0][:],
                        bias[0:1, hp * 128:(hp + 1) * 128],
                        ones[0:1, :],
                        start=False, stop=True,
                    )
                def evict():
                    dst = dest[:, sb * 512:(sb + 1) * 512]
                    if on_act:
                        nc.scalar.activation(dst, ps_box[0][:], Copy)
                    else:
                        nc.vector.tensor_copy(dst, ps_box[0][:])
                return [mk_mm(kd) for kd in range(NKD)] + [bias_step, evict]

            # hp=0 up front (attention for heads 0/1 needs it)
            for (xts, wts, bias, dest) in (
                (xtq_ts, wqt, bqt, qts[0]),
                (xtk_ts, wkt, bkt, kts[0]),
            ):
                for sb in range(NSB):
                    for step in proj_group_steps(xts, wts, bias, dest, 0, sb, True):
                        step()

            # filler queue: hp=1..3 (evictions on DVE: ACT paces exp in attention)
            filler = []
            for hp in range(1, NHP):
                for (xts, wts, bias, dest) in (
                    (xtq_ts, wqt, bqt, qts[hp]),
                    (xtk_ts, wkt, bkt, kts[hp]),
                ):
                    for sb in range(NSB):
                        steps = proj_group_steps(xts, wts, bias, dest, hp, sb, False)
                        def group(steps=steps):
                            for st_fn in steps:
                                st_fn()
                        filler.append(group)
            filler.reverse()  # pop() from the front

            ots = [otp.tile([128, S], bf, name=f"ot{i}", tag="ot") for i in range(NHP)]

            def yproj_group(st, eb):
                def group():
                    ps = psp.tile([128, 512], f32, name="psy", tag="ps")
                    for hp in range(NHP):
                        nc.tensor.matmul(
                            ps[:],
                            ots[hp][:, st * 128:(st + 1) * 128],
                            wot[hp][:, eb * 512:(eb + 1) * 512],
                            start=(hp == 0), stop=(hp == NHP - 1),
                        )
                    ys = ysp.tile([128, 512], f32, name="ys", tag="ys")
                    nc.vector.tensor_copy(ys[:], ps[:])
                    nc.sync.dma_start(
                        y.ap()[st * 128:(st + 1) * 128, eb * 512:(eb + 1) * 512],
                        ys[:],
                    )
                return group

            # ---- attention: scoresT [k, q] chunks of CH k-tiles, AV delayed one
            # chunk (software pipeline) so PE never waits on the exp of the
            # chunk it just scored. Projection/yproj groups are woven in as
            # whole-group filler to keep the tensor engine HAM-warm.
            proj_chunks = sum(len(_chunks(4 * (j + 1))) for j in range(NSB)) * 6
            pace = max(1, proj_chunks // max(1, len(filler)))
            chunk_no = [0]
            yfiller = []

            def maybe_filler(force=False):
                if filler and (force or chunk_no[0] % pace == 0):
                    filler.pop()()
                elif yfiller:
                    yfiller.pop()()
                    if len(yfiller) > 4:
                        yfiller.pop()()

            def attend(h, j):
                hp, sub = h // 2, h % 2
                base = sub * 64
                qt_h = qts[hp][base:base + 64, :]
                kt_h = kts[hp][base:base + 64, :]
                av = psp.tile([128, 512], f32, name="av", tag="ps")
                nkt = 4 * (j + 1)
                prev_av = None

                def make_av(ex, c0, cn):
                    def emit():
                        for r in range(cn):
                            kti = c0 + r
                            nc.tensor.matmul(
                                av[0:65, :],
                                vts[kti][:, h * 65:(h + 1) * 65],
                                ex[:, r * 512:(r + 1) * 512],
                                start=(kti == 0), stop=(kti == nkt - 1),
                            )
                    return emit

                for (c0, cn) in _chunks(nkt):
                    sc = scp.tile([128, CH * 512], f32, name="sc")
                    for r in range(cn):
                        kti = c0 + r
                        nc.tensor.matmul(
                            sc[:, r * 512:(r + 1) * 512],
                            kt_h[:, kti * 128:(kti + 1) * 128],
                            qt_h[:, j * 512:(j + 1) * 512],
                            start=True, stop=True,
                        )
                    ex = expool.tile([128, CH * 512], bf, name="ex")
                    nc.scalar.activation(
                        ex[:, 0:cn * 512], sc[:, 0:cn * 512], Exp,
                        scale=float(SCALE))
                    for r in range(cn):
                        rr = (c0 + r) - 4 * j
                        if rr >= 0:   # diagonal k-tile: causal mask multiply
                            nc.vector.tensor_mul(
                                ex[:, r * 512:(r + 1) * 512],
                                ex[:, r * 512:(r + 1) * 512],
                                dmt[:, rr * 512:(rr + 1) * 512],
                            )
                    chunk_no[0] += 1
                    maybe_filler()
                    if prev_av is not None:
                        prev_av()
                    prev_av = make_av(ex, c0, cn)
                maybe_filler(force=False)
                prev_av()
                # evict av to SBUF (frees PSUM slot), then normalize:
                # denom -> [128,4] reshape -> fast recip -> bcast -> multiply
                avs = ysp.tile([65, 512], f32, name="avs", tag="ys")
                nc.vector.tensor_copy(avs[:], av[0:65, :])
                rsh = rcp.tile([128, 4], f32, name="rsh", tag="rsh")
                nc.sync.dma_start(rsh[:], avs[64:65, :])
                rr_t = rcp.tile([128, 4], f32, name="rr", tag="rr")
                nc.vector.reciprocal(rr_t[:], rsh[:])
                rrow = rcp.tile([1, 512], f32, name="rrow", tag="rrow")
                nc.sync.dma_start(rrow[:], rr_t[:])
                rb = rbp.tile([64, 512], f32, name="rb", tag="rb")
                nc.gpsimd.partition_broadcast(rb[:], rrow[:], channels=64)
                nc.vector.tensor_mul(
                    ots[hp][base:base + 64, j * 512:(j + 1) * 512],
                    avs[0:64, :],
                    rb[:],
                )

            for h in range(6):
                for j in range(NSB):
                    attend(h, j)
            # last head pair: j-major so finished yproj tiles fill the gaps
            for j in range(NSB):
                attend(6, j)
                attend(7, j)
                for st in range(4 * j, 4 * j + 4):
                    for eb in range(2):
                        yfiller.append(yproj_group(st, eb))
            while filler:
                filler.pop()()
            while yfiller:
                yfiller.pop()()

    nc.compile()
    return nc


def _diag_mask():
    tri = np.triu(np.ones((128, 128), np.float32))  # mask[k,q]=1 iff k<=q
    m = np.ones((128, 2048), np.float32)
    for r in range(4):
        m[:, r * 512:r * 512 + r * 128] = 0.0
        m[:, r * 512 + r * 128:r * 512 + (r + 1) * 128] = tri
    return m.astype(BF16)


def _shard_inputs(q_in, k_in, v_in, Wq, bq, Wk, bk, Wv, bv, Wo, bo):
    dm = _diag_mask()
    in_maps = []
    for core in range(NCORES):
        b, g = core // 2, core % 2
        cs = slice(g * DPC, (g + 1) * DPC)
        in_maps.append({
            "xtq": np.ascontiguousarray(q_in[b].T).astype(BF16),
            "xtk": np.ascontiguousarray(k_in[b].T).astype(BF16),
            "xtv": np.ascontiguousarray(v_in[b].T).astype(BF16),
            "wq": Wq[:, cs].astype(BF16),
            "wk": Wk[:, cs].astype(BF16),
            "wv": Wv[:, cs].astype(BF16),
            "wo": np.ascontiguousarray(Wo[cs, :]).astype(BF16),
            "bq": bq[cs].reshape(1, DPC).astype(BF16),
            "bk": bk[cs].reshape(1, DPC).astype(BF16),
            "bv": bv[cs].reshape(1, DPC).astype(BF16),
            "dmask": dm,
        })
    return in_maps


def kernel(q_in, k_in, v_in, Wq, bq, Wk, bk, Wv, bv, Wo, bo, _trace=False):
    from concourse.bass_utils import run_bass_kernel_spmd

    global _compiled
    if _compiled is None:
        _compiled = _build()

    args = [np.asarray(a, np.float32) for a in
            (q_in, k_in, v_in, Wq, bq, Wk, bk, Wv, bv, Wo, bo)]
    in_maps = _shard_inputs(*args)
    res = run_bass_kernel_spmd(
        _compiled, in_maps, core_ids=list(range(NCORES)), trace=_trace,
    )
    bo_f = args[10]
    out = np.empty((B, S, D), np.float32)
    for b in range(B):
        out[b] = res.results[2 * b]["y"] + res.results[2 * b + 1]["y"] + bo_f
    if _trace:
        kernel.last_results = res
    return out


# revision 15
# speedup vs baseline: 1.1398x; 1.1398x over previous
"""Multi-head attention (B=4, S=2048, D=1024, H=16, causal) on 8 TRN2 NeuronCores.

Sharding: core i handles batch i//2 and head-group i%2 (8 heads / 512 projection
columns). Each core computes a partial output projection over its 512 rows of Wo;
the host sums the two partials per batch and adds bo. No device collectives.

Per-core dataflow (bf16 matmuls, fp32 softmax):
  QT/KT = W-stationary projections of pre-transposed x; V in natural layout with
  an interleaved ones column per head (softmax denominator rides the AV matmul).
  Scores are computed transposed [k, q] in 3-k-tile PSUM chunks; one wide ACT
  exp per chunk evicts to SBUF bf16; causal masking is a single multiply per
  diagonal k-tile against a host-provided mask; AV accumulates [out^T | denom];
  normalization uses a DMA-reshaped reciprocal ([1,512] -> [128,4] so the DVE
  divides 4 elements per lane instead of 512) and a GPSIMD partition broadcast.
"""

import sys

for _p in ("/opt/trn_rl_repo",):
    if _p not in sys.path:
        sys.path.insert(0, _p)

import numpy as np
import ml_dtypes

BF16 = ml_dtypes.bfloat16

B, S, D = 4, 2048, 1024
H, HD = 16, 64
HPC = H // 2          # heads per core: 8
DPC = D // 2          # projection cols per core: 512
NCORES = 8
SCALE = 1.0 / np.sqrt(np.float32(HD))
CH = 3                # k-tiles per score chunk (3 PSUM banks, double buffered)

_compiled = None


def _chunks(nkt):
    out, s = [], 0
    while s < nkt:
        n = min(CH, nkt - s)
        out.append((s, n))
        s += n
    return out


def _build():
    import concourse.bacc as bacc
    import concourse.mybir as mybir
    import concourse.tile as tile

    f32 = mybir.dt.float32
    bf = mybir.dt.bfloat16
    Exp = mybir.ActivationFunctionType.Exp
    Copy = mybir.ActivationFunctionType.Copy

    nc = bacc.Bacc("TRN2", target_bir_lowering=False, debug=False)

    xtq = nc.dram_tensor("xtq", [D, S], bf, kind="ExternalInput")
    xtk = nc.dram_tensor("xtk", [D, S], bf, kind="ExternalInput")
    xtv = nc.dram_tensor("xtv", [D, S], bf, kind="ExternalInput")
    wq = nc.dram_tensor("wq", [D, DPC], bf, kind="ExternalInput")
    wk = nc.dram_tensor("wk", [D, DPC], bf, kind="ExternalInput")
    wv = nc.dram_tensor("wv", [D, DPC], bf, kind="ExternalInput")
    wo = nc.dram_tensor("wo", [DPC, D], bf, kind="ExternalInput")
    bq = nc.dram_tensor("bq", [1, DPC], bf, kind="ExternalInput")
    bk = nc.dram_tensor("bk", [1, DPC], bf, kind="ExternalInput")
    bv = nc.dram_tensor("bv", [1, DPC], bf, kind="ExternalInput")
    dmask = nc.dram_tensor("dmask", [128, 2048], bf, kind="ExternalInput")
    y = nc.dram_tensor("y", [S, D], f32, kind="ExternalOutput")

    NKD = D // 128        # 8 contraction tiles for projections
    NST = S // 128        # 16 seq tiles
    NSB = S // 512        # 4 seq blocks
    NHP = HPC // 2        # 4 head pairs / 128-wide col groups

    with tile.TileContext(nc) as tc:
        with (
            tc.tile_pool(name="consts", bufs=1) as consts,
            tc.tile_pool(name="wqp", bufs=NKD) as wqp,
            tc.tile_pool(name="wkp", bufs=NKD) as wkp,
            tc.tile_pool(name="wvp", bufs=NKD) as wvp,
            tc.tile_pool(name="wop", bufs=4) as wop,
            tc.tile_pool(name="xt", bufs=2 * NKD) as xtp,
            tc.tile_pool(name="qt", bufs=NHP) as qtp,
            tc.tile_pool(name="kt", bufs=NHP) as ktp,
            tc.tile_pool(name="vp", bufs=NST) as vpool,
            tc.tile_pool(name="ex", bufs=3) as expool,
            tc.tile_pool(name="ot", bufs=NHP) as otp,
            tc.tile_pool(name="ys", bufs=2) as ysp,
            tc.tile_pool(name="rb", bufs=1) as rbp,
            tc.tile_pool(name="rc", bufs=1) as rcp,
            tc.tile_pool(name="ps", bufs=2, space="PSUM") as psp,
            tc.tile_pool(name="sc", bufs=2, space="PSUM") as scp,
        ):
            # constants
            dmt = consts.tile([128, 2048], bf, tag="dmt")
            nc.sync.dma_start(dmt[:], dmask.ap()[:])
            ones = consts.tile([1, 512], bf, tag="ones")
            nc.gpsimd.memset(ones[:], 1.0)
            bqt = consts.tile([1, DPC], bf, tag="bq")
            nc.sync.dma_start(bqt[:], bq.ap()[:])
            bkt = consts.tile([1, DPC], bf, tag="bk")
            nc.sync.dma_start(bkt[:], bk.ap()[:])
            bvt = consts.tile([1, DPC], bf, tag="bv")
            nc.sync.dma_start(bvt[:], bv.ap()[:])

            # weights: wv + xtv queued first so the first V matmul starts ASAP
            wvt = []
            for kd in range(NKD):
                w = wvp.tile([128, DPC], bf, name=f"wv{kd}", tag="wv")
                nc.sync.dma_start(w[:], wv.ap()[kd * 128:(kd + 1) * 128, :])
                wvt.append(w)
            vts = []
            xts = []
            for kd in range(NKD):
                xt = xtp.tile([128, S], bf, name=f"xt_v{kd}", tag="xt")
                nc.sync.dma_start(xt[:], xtv.ap()[kd * 128:(kd + 1) * 128, :])
                xts.append(xt)
            wqt, wkt = [], []
            for kd in range(NKD):
                for lst, pool, t in ((wqt, wqp, wq), (wkt, wkp, wk)):
                    w = pool.tile([128, DPC], bf)
                    nc.sync.dma_start(w[:], t.ap()[kd * 128:(kd + 1) * 128, :])
                    lst.append(w)
            wot = []
            for hp in range(4):
                w = wop.tile([128, D], bf)
                nc.sync.dma_start(w[:], wo.ap()[hp * 128:(hp + 1) * 128, :])
                wot.append(w)

            # ---- V projection (natural layout), interleaved [8 heads x 65] + ones col
            for st in range(NST):
                vt = vpool.tile([128, HPC * 65], bf)
                vts.append(vt)
                ps = psp.tile([128, 512], f32, tag="ps")
                for kd in range(NKD):
                    nc.tensor.matmul(
                        ps[:],
                        xts[kd][:, st * 128:(st + 1) * 128],
                        wvt[kd][:],
                        start=(kd == 0), stop=False,
                    )
                nc.tensor.matmul(ps[:], ones[0:1, 0:128], bvt[0:1, :],
                                 start=False, stop=True)
                v3 = vt[:].rearrange("p (h c) -> p h c", h=HPC, c=65)
                nc.scalar.activation(
                    v3[:, :, 0:64],
                    ps[:].rearrange("p (h c) -> p h c", h=HPC, c=64),
                    Copy,
                )
                nc.gpsimd.memset(v3[:, :, 64:65], 1.0)

            # ---- QT / KT projections. hp=0 is emitted up front; hp=1..3 are
            # queued as fine-grained filler steps woven between attention chunks
            # so the tensor engine never idles below the HAM busy threshold
            # while ACT paces the exp pipeline.
            qts, kts = [], []
            xtq_ts, xtk_ts = [], []
            for (src_t, xlist) in ((xtq, xtq_ts), (xtk, xtk_ts)):
                for kd in range(NKD):
                    xt = xtp.tile([128, S], bf, name=f"xt_{src_t.name}{kd}", tag="xt")
                    nc.sync.dma_start(xt[:], src_t.ap()[kd * 128:(kd + 1) * 128, :])
                    xlist.append(xt)
            for pool, lst, nm in ((qtp, qts, "qt"), (ktp, kts, "kt")):
                for hp in range(NHP):
                    lst.append(pool.tile([128, S], bf, name=f"{nm}{hp}", tag=nm))

            def proj_group_steps(xts, wts, bias, dest, hp, sb, on_act):
                ps_box = []
                def mk_mm(kd):
                    def step():
                        if kd == 0:
                            ps_box.append(psp.tile([128, 512], f32, name="psq", tag="ps"))
                        nc.tensor.matmul(
                            ps_box[0][:],
                            wts[kd][:, hp * 128:(hp + 1) * 128],
                            xts[kd][:, sb * 512:(sb + 1) * 512],
                            start=(kd == 0), stop=False,
                        )
                    return step
                def bias_step():
                    nc.tensor.matmul(
                        ps_box[0][:],
                        bias[0:1, hp * 128:(hp + 1) * 128],
                        ones[0:1, :],
                        start=False, stop=True,
                    )
                def evict():
                    dst = dest[:, sb * 512:(sb + 1) * 512]
                    if on_act:
                        nc.scalar.activation(dst, ps_box[0][:], Copy)
                    else:
                        nc.vector.tensor_copy(dst, ps_box[0][:])
                return [mk_mm(kd) for kd in range(NKD)] + [bias_step, evict]

            # hp=0 up front (attention for heads 0/1 needs it)
            for (xts, wts, bias, dest) in (
                (xtq_ts, wqt, bqt, qts[0]),
                (xtk_ts, wkt, bkt, kts[0]),
            ):
                for sb in range(NSB):
                    for step in proj_group_steps(xts, wts, bias, dest, 0, sb, True):
                        step()

            # filler queue: hp=1..3 (evictions on DVE: ACT paces exp in attention)
            filler = []
            for hp in range(1, NHP):
                for (xts, wts, bias, dest) in (
                    (xtq_ts, wqt, bqt, qts[hp]),
                    (xtk_ts, wkt, bkt, kts[hp]),
                ):
                    for sb in range(NSB):
                        steps = proj_group_steps(xts, wts, bias, dest, hp, sb, False)
                        def group(steps=steps):
                            for st_fn in steps:
                                st_fn()
                        filler.append(group)
            filler.reverse()  # pop() from the front

            ots = [otp.tile([128, S], bf, name=f"ot{i}", tag="ot") for i in range(NHP)]

            def yproj_group(st, eb):
                def group():
                    ps = psp.tile([128, 512], f32, name="psy", tag="ps")
                    for hp in range(NHP):
                        nc.tensor.matmul(
                            ps[:],
                            ots[hp][:, st * 128:(st + 1) * 128],
                            wot[hp][:, eb * 512:(eb + 1) * 512],
                            start=(hp == 0), stop=(hp == NHP - 1),
                        )
                    ys = ysp.tile([128, 512], f32, name="ys", tag="ys")
                    nc.vector.tensor_copy(ys[:], ps[:])
                    nc.sync.dma_start(
                        y.ap()[st * 128:(st + 1) * 128, eb * 512:(eb + 1) * 512],
                        ys[:],
                    )
                return group

            # ---- attention: scoresT [k, q] chunks of CH k-tiles, AV delayed one
            # chunk (software pipeline) so PE never waits on the exp of the
            # chunk it just scored. Projection/yproj groups are woven in as
            # whole-group filler to keep the tensor engine HAM-warm.
            proj_chunks = sum(len(_chunks(4 * (j + 1))) for j in range(NSB)) * 6
            pace = max(1, proj_chunks // max(1, len(filler)))
            chunk_no = [0]
            yfiller = []

            def maybe_filler(force=False):
                if filler and (force or chunk_no[0] % pace == 0):
                    filler.pop()()
                elif yfiller:
                    yfiller.pop()()
                    if len(yfiller) > 4:
                        yfiller.pop()()

            def attend(h, j):
                hp, sub = h // 2, h % 2
                base = sub * 64
                qt_h = qts[hp][base:base + 64, :]
                kt_h = kts[hp][base:base + 64, :]
                av = psp.tile([128, 512], f32, name="av", tag="ps")
                nkt = 4 * (j + 1)
                prev_av = None

                def make_av(ex, c0, cn):
                    def emit():
                        for r in range(cn):
                            kti = c0 + r
                            nc.tensor.matmul(
                                av[0:65, :],
                                vts[kti][:, h * 65:(h + 1) * 65],
                                ex[:, r * 512:(r + 1) * 512],
                                start=(kti == 0), stop=(kti == nkt - 1),
                            )
                    return emit

                for (c0, cn) in _chunks(nkt):
                    sc = scp.tile([128, CH * 512], f32, name="sc")
                    for r in range(cn):
                        kti = c0 + r
                        nc.tensor.matmul(
                            sc[:, r * 512:(r + 1) * 512],
                            kt_h[:, kti * 128:(kti + 1) * 128],
                            qt_h[:, j * 512:(j + 1) * 512],
                            start=True, stop=True,
                        )
                    ex = expool.tile([128, CH * 512], bf, name="ex")
                    nc.scalar.activation(
                        ex[:, 0:cn * 512], sc[:, 0:cn * 512], Exp,
                        scale=float(SCALE))
                    for r in range(cn):
                        rr = (c0 + r) - 4 * j
                        if rr >= 0:   # diagonal k-tile: causal mask multiply
                            nc.vector.tensor_mul(
                                ex[:, r * 512:(r + 1) * 512],
                                ex[:, r * 512:(r + 1) * 512],
                                dmt[:, rr * 512:(rr + 1) * 512],
                            )
                    chunk_no[0] += 1
                    maybe_filler()
                    if prev_av is not None:
                        prev_av()
                    prev_av = make_av(ex, c0, cn)
                maybe_filler(force=False)
                prev_av()
                # evict av to SBUF (frees PSUM slot), then normalize:
                # denom -> [128,4] reshape -> fast recip -> bcast -> multiply
                avs = ysp.tile([65, 512], f32, name="avs", tag="ys")
                nc.vector.tensor_copy(avs[:], av[0:65, :])
                rsh = rcp.tile([128, 4], f32, name="rsh", tag="rsh")
                nc.sync.dma_start(rsh[:], avs[64:65, :])
                rr_t = rcp.tile([128, 4], f32, name="rr", tag="rr")
                nc.vector.reciprocal(rr_t[:], rsh[:])
                rrow = rcp.tile([1, 512], f32, name="rrow", tag="rrow")
                nc.sync.dma_start(rrow[:], rr_t[:])
                rb = rbp.tile([64, 512], f32, name="rb", tag="rb")
                nc.gpsimd.partition_broadcast(rb[:], rrow[:], channels=64)
                nc.vector.tensor_mul(
                    ots[hp][base:base + 64, j * 512:(j + 1) * 512],
                    avs[0:64, :],
                    rb[:],
                )

            for h in range(6):
                for j in range(NSB):
                    attend(h, j)
            # last head pair: j-major so finished yproj tiles fill the gaps
            for j in range(NSB):
                attend(6, j)
                attend(7, j)
                for st in range(4 * j, 4 * j + 4):
                    for eb in range(2):
                        yfiller.append(yproj_group(st, eb))
            while filler:
                filler.pop()()
            while yfiller:
                yfiller.pop()()

    nc.compile()
    return nc


def _diag_mask():
    tri = np.triu(np.ones((128, 128), np.float32))  # mask[k,q]=1 iff k<=q
    m = np.ones((128, 2048), np.float32)
    for r in range(4):
        m[:, r * 512:r * 512 + r * 128] = 0.0
        m[:, r * 512 + r * 128:r * 512 + (r + 1) * 128] = tri
    return m.astype(BF16)


def _shard_inputs(q_in, k_in, v_in, Wq, bq, Wk, bk, Wv, bv, Wo, bo):
    dm = _diag_mask()
    in_maps = []
    for core in range(NCORES):
        b, g = core // 2, core % 2
        cs = slice(g * DPC, (g + 1) * DPC)
        in_maps.append({
            "xtq": np.ascontiguousarray(q_in[b].T).astype(BF16),
            "xtk": np.ascontiguousarray(k_in[b].T).astype(BF16),
            "xtv": np.ascontiguousarray(v_in[b].T).astype(BF16),
            "wq": Wq[:, cs].astype(BF16),
            "wk": Wk[:, cs].astype(BF16),
            "wv": Wv[:, cs].astype(BF16),
            "wo": np.ascontiguousarray(Wo[cs, :]).astype(BF16),
            "bq": bq[cs].reshape(1, DPC).astype(BF16),
            "bk": bk[cs].reshape(1, DPC).astype(BF16),
            "bv": bv[cs].reshape(1, DPC).astype(BF16),
            "dmask": dm,
        })
    return in_maps


def kernel(q_in, k_in, v_in, Wq, bq, Wk, bk, Wv, bv, Wo, bo, _trace=False):
    from concourse.bass_utils import run_bass_kernel_spmd

    global _compiled
    if _compiled is None:
        _compiled = _build()

    args = [np.asarray(a, np.float32) for a in
            (q_in, k_in, v_in, Wq, bq, Wk, bk, Wv, bv, Wo, bo)]
    in_maps = _shard_inputs(*args)
    res = run_bass_kernel_spmd(
        _compiled, in_maps, core_ids=list(range(NCORES)), trace=_trace,
    )
    bo_f = args[10]
    out = np.empty((B, S, D), np.float32)
    for b in range(B):
        out[b] = res.results[2 * b]["y"] + res.results[2 * b + 1]["y"] + bo_f
    if _trace:
        kernel.last_results = res
    return out
